# revision 35
# baseline (speedup 1.0000x reference)
"""Multi-head self-attention (B=2, N=2048, D=1024, H=16, dh=64) on 8 trn2 cores.

Sharding: core c -> batch b = c // 4, head-group hg = c % 4 (4 heads per core).
Each core computes partial = Attn_{heads hg}(x_b) @ Wo[rows hg]; the host sums
the 4 partials per batch and adds bo (the unshard step).

Per-core pipeline (matmuls in f32r = full-rate fp32 on the PE):
  1. PE-transpose x_b -> xT (D on partitions), interleaved with per-group
     q/k projections for pair 0 and v tiles 0..3
  2. attention per head pair (2 heads packed in disjoint PE row groups so the
     score matmuls can run concurrently on silicon), per query block of 512,
     per key tile of 128:
       scoresT (2 MMs) -> exp on ScalarE (scale folded in, one [128,1024]
       instr covering both heads) -> attn@v accumulation in PSUM (v carries a
       ones column per head emitting softmax denominators on psum row 64)
     Pair-0 window is filled with v tiles 4..15 + pair-1 q/k projections;
     pair-1 window is filled with the output projection of completed query
     blocks.  Denominator reciprocals on DVE, broadcast to 128 partitions on
     GPSIMD, ctxT normalized in place on GPSIMD.
  3. out tiles = ctxT.T @ Wo via 2-step PSUM accumulation, DVE copy, DMA out.
"""

import numpy as np

B, N, D = 2, 2048, 1024
H, DH = 16, 64
HPC = 4                # heads per core
CS = HPC * DH          # 256 = per-core slice of the inner dim
NCORES = 8
SCALE = DH ** -0.5

NT = N // 128          # 16 token tiles
KT = D // 128          # 8 contraction tiles
NIB = N // 512         # 4 query blocks
VW = DH + 1            # 65: v plus ones column

_CACHE = {}


def _build_nc(reps=1):
    import concourse.bass as bass
    import concourse.bacc as bacc
    import concourse.mybir as mybir
    import concourse.tile as tile
    from contextlib import ExitStack

    f32 = mybir.dt.float32
    f32r = mybir.dt.float32r
    bf16 = mybir.dt.bfloat16
    PSUM = bass.MemorySpace.PSUM
    Exp = mybir.ActivationFunctionType.Exp

    nc = bacc.Bacc()

    x_d = nc.dram_tensor("x", [N, D], bf16, kind="ExternalInput")
    wq_d = nc.dram_tensor("wq", [D, CS], bf16, kind="ExternalInput")
    wk_d = nc.dram_tensor("wk", [D, CS], bf16, kind="ExternalInput")
    wv_d = nc.dram_tensor("wv", [D, CS], bf16, kind="ExternalInput")
    wo_d = nc.dram_tensor("wo", [CS, D], f32r, kind="ExternalInput")
    out_d = nc.dram_tensor("out", [N, D], f32, kind="ExternalOutput")
    den_d = nc.dram_tensor("den_scratch", [16, 512], f32r)

    import ml_dtypes as _mld
    ident_d = nc.inline_tensor(np.eye(128).astype(_mld.bfloat16), name="ident")

    # grouped views for batched DMAs
    x_g = x_d.rearrange("(g j p) d -> g p j d", p=128, j=2)        # [8][128,2,1024]
    wq_g = wq_d.rearrange("(k p) c -> p k c", p=128)               # [128,8,256]
    wk_g = wk_d.rearrange("(k p) c -> p k c", p=128)
    wv_g = wv_d.rearrange("(k p) c -> p k c", p=128)
    wo_g = wo_d.rearrange("(k p) c -> p k c", p=128)               # [128,2,1024]
    out_g1 = out_d.rearrange("(q p) e -> q p e", p=128)            # [16][128,1024]

    with tile.TileContext(nc) as tc:
      for _rep in range(reps):
       with ExitStack() as es:
             singles = es.enter_context(tc.tile_pool(name="singles", bufs=1))

             ones4 = singles.tile([128, HPC, 1], bf16, tag="ones4")
             nc.vector.memset(ones4, 1.0)

             wq_sb = singles.tile([128, KT, CS], bf16, tag="wq")
             wk_sb = singles.tile([128, KT, CS], bf16, tag="wk")
             wv_sb = singles.tile([128, KT, CS], bf16, tag="wv")
             wo_sb = singles.tile([128, 2, D], f32r, tag="wo")

             qT = [singles.tile([128, N], f32r, tag=f"qT{p}", name=f"qT{p}") for p in range(2)]
             kTt = [singles.tile([128, N], f32r, tag=f"kT{p}", name=f"kT{p}") for p in range(2)]
             vA = [singles.tile([128, HPC * VW], bf16, tag=f"v{t}", name=f"v{t}") for t in range(NT)]
             ctxT = [singles.tile([128, N], f32r, tag=f"ctxT{p}", name=f"ctxT{p}") for p in range(2)]
             o_sb = es.enter_context(tc.tile_pool(name="osb", bufs=2))
             rec_pool = es.enter_context(tc.tile_pool(name="recp", bufs=2))

             # ---- phase 1: transpose x; interleave pair-0 q/k and v[0:4] ----
             xps = ExitStack()
             xT_pool = xps.enter_context(tc.tile_pool(name="xTp", bufs=1))
             pj_ps = xps.enter_context(tc.tile_pool(name="pjps", bufs=2, space=PSUM))
             xT_all = xT_pool.tile([128, KT, N], bf16, tag="xT", name="xT")
             xT = [xT_all[:, k, :] for k in range(KT)]

             def emit_v(t):
                 pv = pj_ps.tile([128, CS], f32, tag="pp", name="ppv")
                 for k in range(KT):
                     nc.tensor.matmul(
                         pv,
                         xT[k][:, t * 128:(t + 1) * 128],
                         wv_sb[:, k, :],
                         start=(k == 0), stop=(k == KT - 1),
                     )
                 v3 = vA[t].rearrange("p (h c) -> p h c", c=VW)
                 nc.vector.tensor_copy(
                     v3[:, :, 0:DH], pv.rearrange("p (h d) -> p h d", d=DH)
                 )
                 nc.vector.tensor_copy(v3[:, :, DH:VW], ones4)

             pending_pq = {}

             def emit_qk(dst, w_sb, p, ib, half):
                 # half 0/1: contraction tiles 0..3 / 4..7 (split so side ops
                 # stay ~2k cycles each; both halves accumulate into one tile)
                 key = (id(dst), p, ib)
                 if half == 0:
                     pq = pj_ps.tile([128, 512], f32, tag="pp", name="pp")
                     pending_pq[key] = pq
                 else:
                     pq = pending_pq.pop(key)
                 for kk in range(4):
                     k = half * 4 + kk
                     nc.tensor.matmul(
                         pq,
                         w_sb[:, k, p * 128:(p + 1) * 128],
                         xT[k][:, ib * 512:(ib + 1) * 512],
                         start=(k == 0), stop=(k == KT - 1),
                     )
                 if half == 1:
                     nc.vector.tensor_copy(dst[p][:, ib * 512:(ib + 1) * 512], pq)

             with ExitStack() as pes:
                 idp = pes.enter_context(tc.tile_pool(name="idp", bufs=1))
                 x_pool = pes.enter_context(tc.tile_pool(name="xp", bufs=2))
                 tp_ps = pes.enter_context(tc.tile_pool(name="tpps", bufs=2, space=PSUM))

                 ident = idp.tile([128, 128], bf16, tag="ident")
                 nc.sync.dma_start(out=ident, in_=ident_d[:, :])

                 wdma = {
                     0: lambda: nc.sync.dma_start(out=wq_sb[:, 0:4, :], in_=wq_g[:, 0:4, :]),
                     1: lambda: nc.sync.dma_start(out=wq_sb[:, 4:8, :], in_=wq_g[:, 4:8, :]),
                     2: lambda: nc.sync.dma_start(out=wk_sb, in_=wk_g),
                     3: lambda: nc.sync.dma_start(out=wv_sb, in_=wv_g),
                     6: lambda: nc.sync.dma_start(out=wo_sb, in_=wo_g),
                 }
                 # pair-0 projection/v emissions paced so each slot's weight
                 # DMA (fired at g=1..4) has landed well before first use
                 prologue_emits = {
                     1: [lambda: emit_qk(qT, wq_sb, 0, 0, 0),
                         lambda: emit_qk(qT, wq_sb, 0, 0, 1)],
                     3: [lambda: emit_qk(kTt, wk_sb, 0, 0, 0),
                         lambda: emit_qk(kTt, wk_sb, 0, 0, 1),
                         lambda: emit_qk(qT, wq_sb, 0, 1, 0),
                         lambda: emit_qk(qT, wq_sb, 0, 1, 1)],
                     5: [lambda: emit_qk(kTt, wk_sb, 0, 1, 0),
                         lambda: emit_qk(kTt, wk_sb, 0, 1, 1),
                         lambda: emit_qk(qT, wq_sb, 0, 2, 0),
                         lambda: emit_qk(qT, wq_sb, 0, 2, 1),
                         lambda: emit_v(0), lambda: emit_v(1)],
                 }
                 for g in range(NT // 2):  # 8 groups of 2 token tiles
                     xt = x_pool.tile([128, 2, D], bf16, tag="x", name="xt")
                     if g == 0:
                         nc.sync.dma_start(out=xt[:, 0, :], in_=x_g[g][:, 0, :])
                         nc.sync.dma_start(out=xt[:, 1, :], in_=x_g[g][:, 1, :])
                     else:
                         nc.sync.dma_start(out=xt, in_=x_g[g])
                     if g in wdma:
                         wdma[g]()
                     for dh in range(KT // 4):
                         ps = tp_ps.tile([128, 4, 256], bf16, tag="tp", name="tp")
                         for dj in range(4):
                             d = 4 * dh + dj
                             for j in range(2):
                                 nc.tensor.transpose(
                                     ps[:, dj, j * 128:(j + 1) * 128],
                                     xt[:, j, d * 128:(d + 1) * 128],
                                     ident,
                                 )
                         nc.vector.tensor_copy(
                             xT_all[:, 4 * dh:4 * dh + 4, g * 256:(g + 1) * 256],
                             ps,
                         )
                     for op in prologue_emits.get(g, []):
                         op()

             # side work queues: window 0 gets remaining v tiles + pair-1
             # projections; window 1 gets the output projection (pushed
             # per completed query block)
             side_ops = [
                 lambda: emit_qk(kTt, wk_sb, 0, 2, 0),
                 lambda: emit_qk(kTt, wk_sb, 0, 2, 1),
                 lambda: emit_v(2),
                 lambda: emit_v(3),
                 lambda: emit_qk(qT, wq_sb, 0, 3, 0),
                 lambda: emit_qk(qT, wq_sb, 0, 3, 1),
                 lambda: emit_qk(kTt, wk_sb, 0, 3, 0),
                 lambda: emit_qk(kTt, wk_sb, 0, 3, 1),
             ]
             side_ops += [(lambda t=t: emit_v(t)) for t in range(4, NT)]
             for ib in range(NIB):
                 side_ops.append(lambda ib=ib: emit_qk(qT, wq_sb, 1, ib, 0))
                 side_ops.append(lambda ib=ib: emit_qk(qT, wq_sb, 1, ib, 1))
                 side_ops.append(lambda ib=ib: emit_qk(kTt, wk_sb, 1, ib, 0))
                 side_ops.append(lambda ib=ib: emit_qk(kTt, wk_sb, 1, ib, 1))

             def emit_out(it, eh):
                 # out tile [128 tok, 512 D] for token tile `it`, D half `eh`
                 ehs = slice(eh * 512, (eh + 1) * 512)
                 po = pj_ps.tile([128, 512], f32, tag="pp", name="po")
                 for cp in range(2):
                     nc.tensor.matmul(
                         po,
                         ctxT[cp][:, it * 128:(it + 1) * 128],
                         wo_sb[:, cp, ehs],
                         start=(cp == 0), stop=(cp == 1),
                     )
                 ot = ot_tiles[it]
                 nc.vector.tensor_copy(ot[:, ehs], po)

             ot_tiles = {}

             def flush_out(q):
                 nc.sync.dma_start(out=out_g1[q], in_=ot_tiles[q])

             # ---- attention ----
             with ExitStack() as aes:
                 sc_ps = aes.enter_context(tc.tile_pool(name="scps", bufs=2, space=PSUM))
                 ctx_ps = aes.enter_context(tc.tile_pool(name="ctxps", bufs=2, space=PSUM))
                 exp_pool = aes.enter_context(tc.tile_pool(name="expp", bufs=6))
                 bc_pool = aes.enter_context(tc.tile_pool(name="bcp", bufs=2))

                 out_q = []  # deferred out-projection ops (window 1)

                 for p in range(2):
                     lh0, lh1 = 2 * p, 2 * p + 1
                     for ib in range(NIB):
                         ibs = slice(ib * 512, (ib + 1) * 512)
                         c0 = ctx_ps.tile([65, 512], f32, tag="ctx", name="ctx0")
                         c1 = ctx_ps.tile([65, 512], f32, tag="ctx", name="ctx1")

                         def av(jt, e):
                             nc.tensor.matmul(
                                 c0,
                                 vA[jt][:, lh0 * VW:(lh0 + 1) * VW],
                                 e[:, 0, :],
                                 start=(jt == 0), stop=(jt == NT - 1),
                             )
                             nc.tensor.matmul(
                                 c1,
                                 vA[jt][:, lh1 * VW:(lh1 + 1) * VW],
                                 e[:, 1, :],
                                 start=(jt == 0), stop=(jt == NT - 1),
                             )

                         # software-pipelined: av lags exp by `skew` tiles
                         # (deep skew on the first block so attention can
                         # start while prologue projections still stream in)
                         skew = 4 if (p == 0 and ib == 0) else 1
                         fed = 0
                         pend = []
                         for jt in range(NT):
                             js = slice(jt * 128, (jt + 1) * 128)
                             sc = sc_ps.tile([128, 2, 512], f32, tag="sc", name="sc")
                             nc.tensor.matmul(
                                 sc[:, 0, :],
                                 kTt[p][0:64, js],
                                 qT[p][0:64, ibs],
                                 start=True, stop=True,
                             )
                             nc.tensor.matmul(
                                 sc[:, 1, :],
                                 kTt[p][64:128, js],
                                 qT[p][64:128, ibs],
                                 start=True, stop=True,
                             )
                             # pace fill work into the ACT-bound pipeline;
                             # early side ops (k/q tails, v tiles) must stay
                             # ahead of their consumers
                             if p == 0:
                                 n = 0
                                 if ib == 0 and fed < 8:
                                     n = 2
                                 elif side_ops and (len(side_ops) > 16 or jt % 2 == 0):
                                     n = 1
                                 for _ in range(n):
                                     if side_ops:
                                         side_ops.pop(0)()
                                         fed += 1
                             else:
                                 if out_q:
                                     out_q.pop(0)()
                                 if ib == NIB - 1 and out_q:
                                     out_q.pop(0)()
                             e = exp_pool.tile([128, 2, 512], bf16, tag="exp", name="exp")
                             nc.scalar.activation(
                                 e.rearrange("p a b -> p (a b)"),
                                 sc.rearrange("p a b -> p (a b)"),
                                 Exp, scale=SCALE,
                             )
                             pend.append((jt, e))
                             if len(pend) > skew:
                                 av(*pend.pop(0))
                         for t in pend:
                             av(*t)
                         # two-step flush: free c0/c1 fast (unnormalized), then
                         # normalize ctxT in place on GPSIMD once recips land
                         rec2 = rec_pool.tile([33, 512], f32r, tag="rec", name="rec")
                         r0 = rec2[0:1, :]
                         r1 = rec2[32:33, :]
                         with nc.allow_low_precision(reason="f32r softmax denom"):
                             nc.vector.reciprocal(r0, c0[64:65, :])
                             nc.vector.reciprocal(r1, c1[64:65, :])
                         nc.vector.tensor_copy(ctxT[p][0:64, ibs], c0[0:64, :])
                         nc.vector.tensor_copy(ctxT[p][64:128, ibs], c1[0:64, :])
                         ri = 4 * p + ib
                         nc.sync.dma_start(out=den_d[2*ri:2*ri+1, :], in_=r0)
                         nc.sync.dma_start(out=den_d[2*ri+1:2*ri+2, :], in_=r1)
                         bc = bc_pool.tile([128, 512], f32r, tag="bc", name="bc")
                         nc.sync.dma_start(
                             out=bc[0:64, :],
                             in_=den_d[2*ri:2*ri+1, :].to_broadcast((64, 512)))
                         nc.sync.dma_start(
                             out=bc[64:128, :],
                             in_=den_d[2*ri+1:2*ri+2, :].to_broadcast((64, 512)))
                         nc.vector.tensor_mul(ctxT[p][:, ibs], ctxT[p][:, ibs], bc)
                         # queue this block's output projection for window 1
                         # (needs both pairs' ctxT for these tokens)
                         if p == 0:
                             continue
                         for itl in range(4 * ib, 4 * ib + 4):
                             def mk(itl=itl):
                                 ot_tiles[itl] = o_sb.tile(
                                     [128, D], f32, tag="ot", name="ot"
                                 )
                             out_q.append(mk)
                             for eh in range(2):
                                 out_q.append(lambda itl=itl, eh=eh: emit_out(itl, eh))
                             out_q.append(lambda itl=itl: flush_out(itl))
                     if p == 0:
                         while side_ops:
                             side_ops.pop(0)()

                 # tail: drain remaining output projection work
                 while out_q:
                     out_q.pop(0)()

             xps.close()

    nc.compile()
    return nc


def get_nc():
    if "nc" not in _CACHE:
        _CACHE["nc"] = _build_nc()
    return _CACHE["nc"]


def make_in_maps(x, Wq, Wk, Wv, Wo, bo):
    import ml_dtypes
    bf = ml_dtypes.bfloat16
    x = np.ascontiguousarray(np.asarray(x, dtype=np.float32).astype(bf))
    Wq = np.asarray(Wq, dtype=np.float32).astype(bf)
    Wk = np.asarray(Wk, dtype=np.float32).astype(bf)
    Wv = np.asarray(Wv, dtype=np.float32).astype(bf)
    Wo = np.asarray(Wo, dtype=np.float32)
    in_maps = []
    for c in range(NCORES):
        b, hg = c // 4, c % 4
        sl = slice(hg * CS, (hg + 1) * CS)
        in_maps.append({
            "x": x[b],
            "wq": np.ascontiguousarray(Wq[:, sl]),
            "wk": np.ascontiguousarray(Wk[:, sl]),
            "wv": np.ascontiguousarray(Wv[:, sl]),
            "wo": np.ascontiguousarray(Wo[sl, :]),
        })
    return in_maps


def combine_outputs(results, bo):
    outs = [np.asarray(r["out"], dtype=np.float64) for r in results]
    full = np.stack([
        outs[0] + outs[1] + outs[2] + outs[3],
        outs[4] + outs[5] + outs[6] + outs[7],
    ]) + np.asarray(bo, dtype=np.float64)
    return full.astype(np.float32)


def kernel(x, Wq, Wk, Wv, Wo, bo):
    from concourse.bass_utils import run_bass_kernel_spmd

    nc = get_nc()
    in_maps = make_in_maps(x, Wq, Wk, Wv, Wo, bo)
    res = run_bass_kernel_spmd(nc, in_maps, list(range(NCORES)))
    return combine_outputs(res.results, bo)


# revision 37
# speedup vs baseline: 1.0205x; 1.0205x over previous
"""Multi-head self-attention (B=2, N=2048, D=1024, H=16, dh=64) on 8 trn2 cores.

Sharding: core c -> batch b = c // 4, head-group hg = c % 4 (4 heads per core).
Each core computes partial = Attn_{heads hg}(x_b) @ Wo[rows hg]; the host sums
the 4 partials per batch and adds bo (the unshard step).

Per-core pipeline (matmuls in f32r = full-rate fp32 on the PE):
  1. PE-transpose x_b -> xT (D on partitions), interleaved with per-group
     q/k projections for pair 0 and v tiles 0..3
  2. attention per head pair (2 heads packed in disjoint PE row groups so the
     score matmuls can run concurrently on silicon), per query block of 512,
     per key tile of 128:
       scoresT (2 MMs) -> exp on ScalarE (scale folded in, one [128,1024]
       instr covering both heads) -> attn@v accumulation in PSUM (v carries a
       ones column per head emitting softmax denominators on psum row 64)
     Pair-0 window is filled with v tiles 4..15 + pair-1 q/k projections;
     pair-1 window is filled with the output projection of completed query
     blocks.  Denominator reciprocals on DVE, broadcast to 128 partitions on
     GPSIMD, ctxT normalized in place on GPSIMD.
  3. out tiles = ctxT.T @ Wo via 2-step PSUM accumulation, DVE copy, DMA out.
"""

import numpy as np

B, N, D = 2, 2048, 1024
H, DH = 16, 64
HPC = 4                # heads per core
CS = HPC * DH          # 256 = per-core slice of the inner dim
NCORES = 8
SCALE = DH ** -0.5

NT = N // 128          # 16 token tiles
KT = D // 128          # 8 contraction tiles
NIB = N // 512         # 4 query blocks
VW = DH + 1            # 65: v plus ones column

_CACHE = {}


def _build_nc(reps=1):
    import concourse.bass as bass
    import concourse.bacc as bacc
    import concourse.mybir as mybir
    import concourse.tile as tile
    from contextlib import ExitStack

    f32 = mybir.dt.float32
    f32r = mybir.dt.float32r
    bf16 = mybir.dt.bfloat16
    PSUM = bass.MemorySpace.PSUM
    Exp = mybir.ActivationFunctionType.Exp

    nc = bacc.Bacc()

    x_d = nc.dram_tensor("x", [N, D], bf16, kind="ExternalInput")
    wq_d = nc.dram_tensor("wq", [D, CS], bf16, kind="ExternalInput")
    wk_d = nc.dram_tensor("wk", [D, CS], bf16, kind="ExternalInput")
    wv_d = nc.dram_tensor("wv", [D, CS], bf16, kind="ExternalInput")
    wo_d = nc.dram_tensor("wo", [CS, D], f32r, kind="ExternalInput")
    out_d = nc.dram_tensor("out", [N, D], f32, kind="ExternalOutput")
    den_d = nc.dram_tensor("den_scratch", [16, 512], f32r)

    import ml_dtypes as _mld
    ident_d = nc.inline_tensor(np.eye(128).astype(_mld.bfloat16), name="ident")

    # grouped views for batched DMAs
    x_g = x_d.rearrange("(g j p) d -> g p j d", p=128, j=2)        # [8][128,2,1024]
    wq_g = wq_d.rearrange("(k p) c -> p k c", p=128)               # [128,8,256]
    wk_g = wk_d.rearrange("(k p) c -> p k c", p=128)
    wv_g = wv_d.rearrange("(k p) c -> p k c", p=128)
    wo_g = wo_d.rearrange("(k p) c -> p k c", p=128)               # [128,2,1024]
    out_g1 = out_d.rearrange("(q p) e -> q p e", p=128)            # [16][128,1024]

    with tile.TileContext(nc) as tc:
      for _rep in range(reps):
       with ExitStack() as es:
             singles = es.enter_context(tc.tile_pool(name="singles", bufs=1))

             ones4 = singles.tile([128, HPC, 1], bf16, tag="ones4")
             nc.vector.memset(ones4, 1.0)

             wq_sb = singles.tile([128, KT, CS], bf16, tag="wq")
             wk_sb = singles.tile([128, KT, CS], bf16, tag="wk")
             wv_sb = singles.tile([128, KT, CS], bf16, tag="wv")
             wo_sb = singles.tile([128, 2, D], f32r, tag="wo")

             qT = [singles.tile([128, N], f32r, tag=f"qT{p}", name=f"qT{p}") for p in range(2)]
             kTt = [singles.tile([128, N], f32r, tag=f"kT{p}", name=f"kT{p}") for p in range(2)]
             vA = [singles.tile([128, HPC * VW], bf16, tag=f"v{t}", name=f"v{t}") for t in range(NT)]
             ctxT = [singles.tile([128, N], f32r, tag=f"ctxT{p}", name=f"ctxT{p}") for p in range(2)]
             o_sb = es.enter_context(tc.tile_pool(name="osb", bufs=2))
             rec_pool = es.enter_context(tc.tile_pool(name="recp", bufs=2))

             # ---- phase 1: transpose x; interleave pair-0 q/k and v[0:4] ----
             xps = ExitStack()
             xT_pool = xps.enter_context(tc.tile_pool(name="xTp", bufs=1))
             pj_ps = xps.enter_context(tc.tile_pool(name="pjps", bufs=2, space=PSUM))
             xT_all = xT_pool.tile([128, KT, N], bf16, tag="xT", name="xT")
             xT = [xT_all[:, k, :] for k in range(KT)]

             def emit_v(t):
                 pv = pj_ps.tile([128, CS], f32, tag="pp", name="ppv")
                 for k in range(KT):
                     nc.tensor.matmul(
                         pv,
                         xT[k][:, t * 128:(t + 1) * 128],
                         wv_sb[:, k, :],
                         start=(k == 0), stop=(k == KT - 1),
                     )
                 v3 = vA[t].rearrange("p (h c) -> p h c", c=VW)
                 nc.vector.tensor_copy(
                     v3[:, :, 0:DH], pv.rearrange("p (h d) -> p h d", d=DH)
                 )
                 nc.vector.tensor_copy(v3[:, :, DH:VW], ones4)

             pending_pq = {}

             def emit_qk(dst, w_sb, p, ib, half):
                 # half 0/1: contraction tiles 0..3 / 4..7 (split so side ops
                 # stay ~2k cycles each; both halves accumulate into one tile)
                 key = (id(dst), p, ib)
                 if half == 0:
                     pq = pj_ps.tile([128, 512], f32, tag="pp", name="pp")
                     pending_pq[key] = pq
                 else:
                     pq = pending_pq.pop(key)
                 for kk in range(4):
                     k = half * 4 + kk
                     nc.tensor.matmul(
                         pq,
                         w_sb[:, k, p * 128:(p + 1) * 128],
                         xT[k][:, ib * 512:(ib + 1) * 512],
                         start=(k == 0), stop=(k == KT - 1),
                     )
                 if half == 1:
                     nc.vector.tensor_copy(dst[p][:, ib * 512:(ib + 1) * 512], pq)

             with ExitStack() as pes:
                 idp = pes.enter_context(tc.tile_pool(name="idp", bufs=1))
                 x_pool = pes.enter_context(tc.tile_pool(name="xp", bufs=2))
                 tp_ps = pes.enter_context(tc.tile_pool(name="tpps", bufs=2, space=PSUM))

                 ident = idp.tile([128, 128], bf16, tag="ident")
                 nc.sync.dma_start(out=ident, in_=ident_d[:, :])

                 wdma = {
                     0: lambda: nc.sync.dma_start(out=wq_sb[:, 0:4, :], in_=wq_g[:, 0:4, :]),
                     1: lambda: nc.sync.dma_start(out=wq_sb[:, 4:8, :], in_=wq_g[:, 4:8, :]),
                     2: lambda: nc.sync.dma_start(out=wk_sb, in_=wk_g),
                     3: lambda: nc.sync.dma_start(out=wv_sb, in_=wv_g),
                     6: lambda: nc.sync.dma_start(out=wo_sb, in_=wo_g),
                 }
                 # pair-0 projection/v emissions paced so each slot's weight
                 # DMA (fired at g=1..4) has landed well before first use
                 prologue_emits = {
                     1: [lambda: emit_qk(qT, wq_sb, 0, 0, 0),
                         lambda: emit_qk(qT, wq_sb, 0, 0, 1)],
                     3: [lambda: emit_qk(kTt, wk_sb, 0, 0, 0),
                         lambda: emit_qk(kTt, wk_sb, 0, 0, 1),
                         lambda: emit_qk(qT, wq_sb, 0, 1, 0),
                         lambda: emit_qk(qT, wq_sb, 0, 1, 1)],
                     5: [lambda: emit_qk(kTt, wk_sb, 0, 1, 0),
                         lambda: emit_qk(kTt, wk_sb, 0, 1, 1),
                         lambda: emit_qk(qT, wq_sb, 0, 2, 0),
                         lambda: emit_qk(qT, wq_sb, 0, 2, 1),
                         lambda: emit_v(0), lambda: emit_v(1)],
                 }
                 for g in range(NT // 2):  # 8 groups of 2 token tiles
                     xt = x_pool.tile([128, 2, D], bf16, tag="x", name="xt")
                     if g == 0:
                         nc.sync.dma_start(out=xt[:, 0, :], in_=x_g[g][:, 0, :])
                         nc.sync.dma_start(out=xt[:, 1, :], in_=x_g[g][:, 1, :])
                     else:
                         nc.sync.dma_start(out=xt, in_=x_g[g])
                     if g in wdma:
                         wdma[g]()
                     for dh in range(KT // 4):
                         ps = tp_ps.tile([128, 4, 256], bf16, tag="tp", name="tp")
                         for dj in range(4):
                             d = 4 * dh + dj
                             for j in range(2):
                                 nc.tensor.transpose(
                                     ps[:, dj, j * 128:(j + 1) * 128],
                                     xt[:, j, d * 128:(d + 1) * 128],
                                     ident,
                                 )
                         nc.vector.tensor_copy(
                             xT_all[:, 4 * dh:4 * dh + 4, g * 256:(g + 1) * 256],
                             ps,
                         )
                     for op in prologue_emits.get(g, []):
                         op()

             # side work queues: window 0 gets remaining v tiles + pair-1
             # projections; window 1 gets the output projection (pushed
             # per completed query block)
             side_ops = [
                 lambda: emit_qk(kTt, wk_sb, 0, 2, 0),
                 lambda: emit_qk(kTt, wk_sb, 0, 2, 1),
                 lambda: emit_v(2),
                 lambda: emit_v(3),
                 lambda: emit_qk(qT, wq_sb, 0, 3, 0),
                 lambda: emit_qk(qT, wq_sb, 0, 3, 1),
                 lambda: emit_qk(kTt, wk_sb, 0, 3, 0),
                 lambda: emit_qk(kTt, wk_sb, 0, 3, 1),
             ]
             side_ops += [(lambda t=t: emit_v(t)) for t in range(4, NT)]
             for ib in range(NIB):
                 side_ops.append(lambda ib=ib: emit_qk(qT, wq_sb, 1, ib, 0))
                 side_ops.append(lambda ib=ib: emit_qk(qT, wq_sb, 1, ib, 1))
                 side_ops.append(lambda ib=ib: emit_qk(kTt, wk_sb, 1, ib, 0))
                 side_ops.append(lambda ib=ib: emit_qk(kTt, wk_sb, 1, ib, 1))

             def emit_out(it, eh):
                 # out tile [128 tok, 512 D] for token tile `it`, D half `eh`
                 ehs = slice(eh * 512, (eh + 1) * 512)
                 po = pj_ps.tile([128, 512], f32, tag="pp", name="po")
                 for cp in range(2):
                     nc.tensor.matmul(
                         po,
                         ctxT[cp][:, it * 128:(it + 1) * 128],
                         wo_sb[:, cp, ehs],
                         start=(cp == 0), stop=(cp == 1),
                     )
                 ot = ot_tiles[it]
                 nc.vector.tensor_copy(ot[:, ehs], po)

             ot_tiles = {}

             def flush_out(q):
                 nc.sync.dma_start(out=out_g1[q], in_=ot_tiles[q])

             # ---- attention ----
             with ExitStack() as aes:
                 sc_ps = aes.enter_context(tc.tile_pool(name="scps", bufs=2, space=PSUM))
                 ctx_ps = aes.enter_context(tc.tile_pool(name="ctxps", bufs=2, space=PSUM))
                 exp_pool = aes.enter_context(tc.tile_pool(name="expp", bufs=6))
                 bc_pool = aes.enter_context(tc.tile_pool(name="bcp", bufs=2))

                 out_q = []  # deferred out-projection ops (window 1)

                 for p in range(2):
                     lh0, lh1 = 2 * p, 2 * p + 1
                     for ib in range(NIB):
                         ibs = slice(ib * 512, (ib + 1) * 512)
                         c0 = ctx_ps.tile([65, 512], f32, tag="ctx", name="ctx0")
                         c1 = ctx_ps.tile([65, 512], f32, tag="ctx", name="ctx1")

                         def av(jg, e0, e1):
                             # 4 MMs grouped by PSUM bank (c0,c0,c1,c1) so the
                             # PE switches banks half as often (HAM-friendly)
                             for jj in range(2):
                                 nc.tensor.matmul(
                                     c0,
                                     vA[2 * jg + jj][:, lh0 * VW:(lh0 + 1) * VW],
                                     e0[:, jj, :],
                                     start=(jg == 0 and jj == 0),
                                     stop=(2 * jg + jj == NT - 1),
                                 )
                             for jj in range(2):
                                 nc.tensor.matmul(
                                     c1,
                                     vA[2 * jg + jj][:, lh1 * VW:(lh1 + 1) * VW],
                                     e1[:, jj, :],
                                     start=(jg == 0 and jj == 0),
                                     stop=(2 * jg + jj == NT - 1),
                                 )

                         # software-pipelined at 2-ktile granularity: av for
                         # group jg-`skew` runs on PE while ScalarE computes
                         # exp for group jg; scores for one head land in one
                         # 2-bank tile (consecutive same-bank writes)
                         skew = 2 if (p == 0 and ib == 0) else 1
                         fed = 0
                         pend = []
                         for jg in range(NT // 2):
                             scA = sc_ps.tile([128, 2, 512], f32, tag="sc", name="scA")
                             scB = sc_ps.tile([128, 2, 512], f32, tag="sc", name="scB")
                             for jj in range(2):
                                 js = slice((2 * jg + jj) * 128, (2 * jg + jj + 1) * 128)
                                 nc.tensor.matmul(
                                     scA[:, jj, :],
                                     kTt[p][0:64, js],
                                     qT[p][0:64, ibs],
                                     start=True, stop=True,
                                 )
                                 nc.tensor.matmul(
                                     scB[:, jj, :],
                                     kTt[p][64:128, js],
                                     qT[p][64:128, ibs],
                                     start=True, stop=True,
                                 )
                             # pace fill work into the ACT-bound pipeline
                             if p == 0:
                                 n = 0
                                 if ib == 0 and fed < 12:
                                     n = 3
                                 elif ib == 0:
                                     n = 2
                                 elif side_ops and (len(side_ops) > 12 or jg % 2 == 0):
                                     n = 2
                                 elif side_ops:
                                     n = 1
                                 for _ in range(n):
                                     if side_ops:
                                         side_ops.pop(0)()
                                         fed += 1
                             else:
                                 for _ in range(2 if ib == NIB - 1 else 1):
                                     if out_q:
                                         out_q.pop(0)()
                             e0 = exp_pool.tile([128, 2, 512], bf16, tag="exp", name="exp")
                             e1 = exp_pool.tile([128, 2, 512], bf16, tag="exp", name="exp")
                             nc.scalar.activation(
                                 e0.rearrange("p a b -> p (a b)"),
                                 scA.rearrange("p a b -> p (a b)"),
                                 Exp, scale=SCALE,
                             )
                             nc.scalar.activation(
                                 e1.rearrange("p a b -> p (a b)"),
                                 scB.rearrange("p a b -> p (a b)"),
                                 Exp, scale=SCALE,
                             )
                             pend.append((jg, e0, e1))
                             if len(pend) > skew:
                                 av(*pend.pop(0))
                         for t in pend:
                             av(*t)
                         # two-step flush: free c0/c1 fast (unnormalized), then
                         # normalize ctxT in place on GPSIMD once recips land
                         rec2 = rec_pool.tile([33, 512], f32r, tag="rec", name="rec")
                         r0 = rec2[0:1, :]
                         r1 = rec2[32:33, :]
                         with nc.allow_low_precision(reason="f32r softmax denom"):
                             nc.vector.reciprocal(r0, c0[64:65, :])
                             nc.vector.reciprocal(r1, c1[64:65, :])
                         nc.vector.tensor_copy(ctxT[p][0:64, ibs], c0[0:64, :])
                         nc.vector.tensor_copy(ctxT[p][64:128, ibs], c1[0:64, :])
                         ri = 4 * p + ib
                         nc.sync.dma_start(out=den_d[2*ri:2*ri+1, :], in_=r0)
                         nc.sync.dma_start(out=den_d[2*ri+1:2*ri+2, :], in_=r1)
                         bc = bc_pool.tile([128, 512], f32r, tag="bc", name="bc")
                         nc.sync.dma_start(
                             out=bc[0:64, :],
                             in_=den_d[2*ri:2*ri+1, :].to_broadcast((64, 512)))
                         nc.sync.dma_start(
                             out=bc[64:128, :],
                             in_=den_d[2*ri+1:2*ri+2, :].to_broadcast((64, 512)))
                         nc.vector.tensor_mul(ctxT[p][:, ibs], ctxT[p][:, ibs], bc)
                         # queue this block's output projection for window 1
                         # (needs both pairs' ctxT for these tokens)
                         if p == 0:
                             continue
                         for itl in range(4 * ib, 4 * ib + 4):
                             def mk(itl=itl):
                                 ot_tiles[itl] = o_sb.tile(
                                     [128, D], f32, tag="ot", name="ot"
                                 )
                             out_q.append(mk)
                             for eh in range(2):
                                 out_q.append(lambda itl=itl, eh=eh: emit_out(itl, eh))
                             out_q.append(lambda itl=itl: flush_out(itl))
                     if p == 0:
                         while side_ops:
                             side_ops.pop(0)()

                 # tail: drain remaining output projection work
                 while out_q:
                     out_q.pop(0)()

             xps.close()

    nc.compile()
    return nc


def get_nc():
    if "nc" not in _CACHE:
        _CACHE["nc"] = _build_nc()
    return _CACHE["nc"]


def make_in_maps(x, Wq, Wk, Wv, Wo, bo):
    import ml_dtypes
    bf = ml_dtypes.bfloat16
    x = np.ascontiguousarray(np.asarray(x, dtype=np.float32).astype(bf))
    Wq = np.asarray(Wq, dtype=np.float32).astype(bf)
    Wk = np.asarray(Wk, dtype=np.float32).astype(bf)
    Wv = np.asarray(Wv, dtype=np.float32).astype(bf)
    Wo = np.asarray(Wo, dtype=np.float32)
    in_maps = []
    for c in range(NCORES):
        b, hg = c // 4, c % 4
        sl = slice(hg * CS, (hg + 1) * CS)
        in_maps.append({
            "x": x[b],
            "wq": np.ascontiguousarray(Wq[:, sl]),
            "wk": np.ascontiguousarray(Wk[:, sl]),
            "wv": np.ascontiguousarray(Wv[:, sl]),
            "wo": np.ascontiguousarray(Wo[sl, :]),
        })
    return in_maps


def combine_outputs(results, bo):
    outs = [np.asarray(r["out"], dtype=np.float64) for r in results]
    full = np.stack([
        outs[0] + outs[1] + outs[2] + outs[3],
        outs[4] + outs[5] + outs[6] + outs[7],
    ]) + np.asarray(bo, dtype=np.float64)
    return full.astype(np.float32)


def kernel(x, Wq, Wk, Wv, Wo, bo):
    from concourse.bass_utils import run_bass_kernel_spmd

    nc = get_nc()
    in_maps = make_in_maps(x, Wq, Wk, Wv, Wo, bo)
    res = run_bass_kernel_spmd(nc, in_maps, list(range(NCORES)))
    return combine_outputs(res.results, bo)
